# revision 27
# baseline (speedup 1.0000x reference)
"""KAN Fourier-linear kernel for 8 Trainium2 NeuronCores.

y[n,o] = sum_{i,g} C0[o,i,g]*cos(g*x[n,i]) + C1[o,i,g]*sin(g*x[n,i]) + bias[o]

Strategy (data-parallel over n, 4096 rows/core, 2 superpasses of 2048):
  - Seeds g=1..8 computed exactly:
      v   = rne(x*(g/2pi) + 1/8)          (DVE tensor_scalar fp32->int32)
      r   = x - v*(2pi/g)                 (DVE scalar_tensor_tensor fp32)
      S_g = Sin(scale=g, bias=0)(r)       = sin(g*x)   (ACT, arg in [-3.93, 2.36])
      C_g = Sin(scale=g, bias=pi/2)(r)    = cos(g*x)   (ACT, arg in [-2.36, 3.93])
  - g=9..16 as product features on DVE (TT bf16, 2x mode):
      Q_r = S_8*C_r, P_r = C_8*C_r  (r=1..8)
    using sin((8+r)x) = 2*Q_r - sin((8-r)x), cos((8+r)x) = 2*P_r - cos((8-r)x);
    the linear reconstruction is absorbed into the weights on the host.
  - g=8 runs first so products interleave with seeds: per step the PE eats
    4 feature tiles (S_r, C_r, Q_r, P_r) while ACT makes 2 and DVE makes 2,
    keeping every engine below the PE rate.
  - y.T = W'.T @ F via PE (bf16, K=4096 accumulated in PSUM over 32 k-tiles).
  - PSUM drained on ACT (Identity + per-partition bias) to bf16.
  - Host: transpose/shard x, build W' from fouriercoeffs (fp64), assemble y.
"""
import math
import numpy as np
from contextlib import ExitStack

import concourse.bass as bass
import concourse.mybir as mybir
import concourse.tile as tile
from concourse import bacc
from concourse.bass_utils import run_bass_kernel_spmd

import ml_dtypes

N_CORES = 8
N_TOTAL = 32768
N_SHARD = N_TOTAL // N_CORES        # 4096 rows per core
INDIM = 128
OUTDIM = 256
GRID = 16
NFEAT = 32                          # 2*GRID features per i
SP = 4                              # n-superpasses per core
S = N_SHARD // SP                   # 2048 columns per superpass
CH = 512                            # matmul moving chunk (PSUM bank)
TWO_PI = 2.0 * math.pi

FP32 = mybir.dt.float32
BF16 = mybir.dt.bfloat16
I32 = mybir.dt.int32

# feature k-tile order (products interleaved with seeds):
#   kt0 = S_8, kt1 = C_8
#   for r=1..7: kt(4r-2)=S_r, kt(4r-1)=C_r, kt(4r)=Q_r, kt(4r+1)=P_r
#   kt30 = Q_8, kt31 = P_8
KT_S = {8: 0, **{r: 4 * r - 2 for r in range(1, 8)}}
KT_C = {8: 1, **{r: 4 * r - 1 for r in range(1, 8)}}
KT_Q = {**{r: 4 * r for r in range(1, 8)}, 8: 30}
KT_P = {**{r: 4 * r + 1 for r in range(1, 8)}, 8: 31}

_CACHED = {}


def _build():
    if "nc" in _CACHED:
        return _CACHED["nc"]
    nc = bacc.Bacc("TRN2", target_bir_lowering=False, debug=False,
                   num_devices=N_CORES)
    xt_d = nc.dram_tensor("xt", [INDIM, N_SHARD], FP32, kind="ExternalInput").ap()
    w_d = nc.dram_tensor("w", [INDIM, NFEAT * OUTDIM], BF16,
                         kind="ExternalInput").ap()
    bt_d = nc.dram_tensor("bt", [INDIM, 2], FP32, kind="ExternalInput").ap()
    bias_d = nc.dram_tensor("bias", [INDIM, 2], FP32, kind="ExternalInput").ap()
    yt_d = nc.dram_tensor("yt", [OUTDIM, N_SHARD], BF16, kind="ExternalOutput").ap()

    with tile.TileContext(nc) as tc, ExitStack() as ctx:
        cpool = ctx.enter_context(tc.tile_pool(name="const", bufs=1))
        vpool = ctx.enter_context(tc.tile_pool(name="v", bufs=1))
        rpool = ctx.enter_context(tc.tile_pool(name="r", bufs=2))
        base8 = ctx.enter_context(tc.tile_pool(name="base8", bufs=2))
        seedpool = ctx.enter_context(tc.tile_pool(name="seed", bufs=2))
        prodpool = ctx.enter_context(tc.tile_pool(name="prod", bufs=3))
        ypool = ctx.enter_context(tc.tile_pool(name="y", bufs=1))
        ppool = ctx.enter_context(tc.tile_pool(name="psum", bufs=1, space="PSUM"))

        # first x slice leads (longest dependency chain); small consts are
        # quick DIRECT2D transfers right behind it
        xt = cpool.tile([INDIM, N_SHARD], FP32)
        wt = cpool.tile([INDIM, NFEAT * OUTDIM], BF16)
        nc.gpsimd.dma_start(xt[:, 0:512], xt_d[:, 0:512])
        bt = cpool.tile([INDIM, 2], FP32)
        nc.sync.dma_start(bt[:], bt_d[:])
        bias = cpool.tile([INDIM, 2], FP32)
        nc.sync.dma_start(bias[:], bias_d[:])
        nc.sync.dma_start(wt[:, 0:512], w_d[:, 0:512])          # kt 0-1
        nc.sync.dma_start(xt[:, 512:2048], xt_d[:, 512:2048])
        nc.sync.dma_start(wt[:, 512:2048], w_d[:, 512:2048])    # kt 2-7
        nc.sync.dma_start(xt[:, 2048:N_SHARD], xt_d[:, 2048:N_SHARD])
        nc.sync.dma_start(wt[:, 2048:NFEAT * OUTDIM],
                          w_d[:, 2048:NFEAT * OUTDIM])

        def mm_feature(kt, f, psums):
            """Accumulate feature tile f (k-tile kt) into both output halves."""
            for oh in range(2):
                lhsT = wt[:, kt * OUTDIM + oh * 128:kt * OUTDIM + oh * 128 + 128]
                for chi in range(S // CH):
                    nc.tensor.matmul(
                        psums[oh][:, chi * CH:(chi + 1) * CH],
                        lhsT, f[:, chi * CH:(chi + 1) * CH],
                        start=(kt == 0), stop=(kt == NFEAT - 1),
                    )

        for sp in range(SP):
            xs = xt[:, sp * S:(sp + 1) * S]
            psum0 = ppool.tile([128, S], FP32, tag=f"p0{sp % 2}")
            psum1 = ppool.tile([128, S], FP32, tag=f"p1{sp % 2}")
            psums = [psum0, psum1]

            def make_seed(g, sg, cg, nsec=1):
                a = np.float32(g / TWO_PI)
                p = np.float32(TWO_PI / g)
                sec = S // nsec
                for si in range(nsec):
                    sl = slice(si * sec, (si + 1) * sec)
                    v = vpool.tile([INDIM, sec], I32, tag="v")
                    nc.vector.tensor_scalar(v[:], xs[:, sl], float(a), 0.125,
                                            mybir.AluOpType.mult,
                                            mybir.AluOpType.add)
                    r = rpool.tile([INDIM, sec], FP32, tag="r")
                    nc.vector.scalar_tensor_tensor(r[:], v[:], float(-p),
                                                   xs[:, sl],
                                                   mybir.AluOpType.mult,
                                                   mybir.AluOpType.add)
                    nc.scalar.activation(sg[:, sl], r[:],
                                         mybir.ActivationFunctionType.Sin,
                                         bias=bt[:, 0:1], scale=float(g))
                    nc.scalar.activation(cg[:, sl], r[:],
                                         mybir.ActivationFunctionType.Sin,
                                         bias=bt[:, 1:2], scale=float(g))

            def drain(oh, nchunks=1):
                dc = S // nchunks
                for ci in range(nchunks):
                    y = ypool.tile([128, dc], BF16, tag=f"y{oh}{ci}{nchunks}")
                    nc.scalar.activation(y[:], psums[oh][:, ci * dc:(ci + 1) * dc],
                                         mybir.ActivationFunctionType.Identity,
                                         bias=bias[:, oh:oh + 1], scale=1.0)
                    nc.sync.dma_start(
                        yt_d[oh * 128:(oh + 1) * 128,
                             sp * S + ci * dc:sp * S + (ci + 1) * dc], y[:])

            s8 = base8.tile([INDIM, S], BF16, tag="S8")
            c8 = base8.tile([INDIM, S], BF16, tag="C8")
            make_seed(8, s8, c8, nsec=2 if sp == 0 else 1)
            mm_feature(KT_S[8], s8, psums)
            mm_feature(KT_C[8], c8, psums)

            for r_ in range(1, 8):
                sg = seedpool.tile([INDIM, S], BF16, tag="Sr")
                cg = seedpool.tile([INDIM, S], BF16, tag="Cr")
                make_seed(r_, sg, cg)
                mm_feature(KT_S[r_], sg, psums)
                mm_feature(KT_C[r_], cg, psums)
                q = prodpool.tile([INDIM, S], BF16, tag="q")
                nc.vector.tensor_tensor(q[:], s8[:], cg[:],
                                        mybir.AluOpType.mult)
                mm_feature(KT_Q[r_], q, psums)
                pr = prodpool.tile([INDIM, S], BF16, tag="p")
                nc.vector.tensor_tensor(pr[:], c8[:], cg[:],
                                        mybir.AluOpType.mult)
                mm_feature(KT_P[r_], pr, psums)

            q = prodpool.tile([INDIM, S], BF16, tag="q")
            nc.vector.tensor_tensor(q[:], s8[:], c8[:], mybir.AluOpType.mult)
            mm_feature(KT_Q[8], q, psums)
            pr = prodpool.tile([INDIM, S], BF16, tag="p")
            nc.vector.tensor_tensor(pr[:], c8[:], c8[:], mybir.AluOpType.mult)
            # last k-tile: finish oh0, drain it while oh1 finishes
            kt = KT_P[8]
            for oh in range(2):
                lhsT = wt[:, kt * OUTDIM + oh * 128:kt * OUTDIM + oh * 128 + 128]
                for chi in range(S // CH):
                    nc.tensor.matmul(
                        psums[oh][:, chi * CH:(chi + 1) * CH],
                        lhsT, pr[:, chi * CH:(chi + 1) * CH],
                        start=False, stop=True,
                    )
                drain(oh, nchunks=1)

    nc.compile()
    _CACHED["nc"] = nc
    return nc


def _prep_inputs(x: np.ndarray, fouriercoeffs: np.ndarray, bias: np.ndarray):
    xt = np.ascontiguousarray(x.astype(np.float32, copy=False).T)  # (128, 32768)

    fc = fouriercoeffs.astype(np.float64, copy=False)
    c_cos = fc[0]                     # (256 o, 128 i, 16 g): cos coeffs
    c_sin = fc[1]                     # sin coeffs

    # sin((8+r)x) = 2 Q_r - sin((8-r)x)   [sin(0x) = 0]
    # cos((8+r)x) = 2 P_r - cos((8-r)x)   [cos(0x) = 1 -> bias]
    wf = np.zeros((NFEAT, OUTDIM, INDIM), np.float64)
    for g in range(1, 9):
        wf[KT_S[g]] += c_sin[:, :, g - 1]
        wf[KT_C[g]] += c_cos[:, :, g - 1]
    for r in range(1, 9):
        gs = 8 + r
        wf[KT_Q[r]] += 2.0 * c_sin[:, :, gs - 1]
        wf[KT_P[r]] += 2.0 * c_cos[:, :, gs - 1]
        if r < 8:
            wf[KT_S[8 - r]] -= c_sin[:, :, gs - 1]
            wf[KT_C[8 - r]] -= c_cos[:, :, gs - 1]
    bias_eff = bias.astype(np.float64).reshape(OUTDIM).copy()
    bias_eff -= c_cos[:, :, 15].sum(axis=1)                   # cos(0x)*W_c16

    # SBUF weight tile: [i, kt*256 + o] bf16
    w_sb = np.ascontiguousarray(
        wf.transpose(2, 0, 1).reshape(INDIM, NFEAT * OUTDIM)
    ).astype(ml_dtypes.bfloat16)

    bt = np.tile(np.array([[0.0, math.pi / 2.0]], dtype=np.float32), (INDIM, 1))
    bias_sb = np.ascontiguousarray(
        bias_eff.reshape(2, 128).T.astype(np.float32))        # (128, 2)
    return xt, w_sb, bt, bias_sb


def kernel(x: np.ndarray, fouriercoeffs: np.ndarray, bias: np.ndarray,
           _trace: bool = False):
    x = np.asarray(x)
    fouriercoeffs = np.asarray(fouriercoeffs)
    bias = np.asarray(bias)
    orig_shape = x.shape
    x2 = x.reshape(-1, INDIM)
    assert x2.shape == (N_TOTAL, INDIM), x2.shape

    nc = _build()
    xt, w_sb, bt, bias_sb = _prep_inputs(x2, fouriercoeffs, bias)
    in_maps = []
    for c in range(N_CORES):
        in_maps.append({
            "xt": np.ascontiguousarray(xt[:, c * N_SHARD:(c + 1) * N_SHARD]),
            "w": w_sb,
            "bt": bt,
            "bias": bias_sb,
        })
    res = run_bass_kernel_spmd(nc, in_maps, list(range(N_CORES)),
                               trace=_trace)
    yt = np.concatenate([res.results[c]["yt"].astype(np.float32)
                         for c in range(N_CORES)], axis=1)
    y = np.ascontiguousarray(yt.T)
    if _trace:
        kernel._last_result = res
    return y.reshape(*orig_shape[:-1], OUTDIM)


# revision 28
# speedup vs baseline: 1.1781x; 1.1781x over previous
"""KAN Fourier-linear kernel for 8 Trainium2 NeuronCores.

y[n,o] = sum_{i,g} C0[o,i,g]*cos(g*x[n,i]) + C1[o,i,g]*sin(g*x[n,i]) + bias[o]

Strategy (data-parallel over n, 4096 rows/core, 2 superpasses of 2048):
  - Seeds g=1..8 computed exactly:
      v   = rne(x*(g/2pi) + 1/8)          (DVE tensor_scalar fp32->int32)
      r   = x - v*(2pi/g)                 (DVE scalar_tensor_tensor fp32)
      S_g = Sin(scale=g, bias=0)(r)       = sin(g*x)   (ACT, arg in [-3.93, 2.36])
      C_g = Sin(scale=g, bias=pi/2)(r)    = cos(g*x)   (ACT, arg in [-2.36, 3.93])
  - g=9..16 as product features on DVE (TT bf16, 2x mode):
      Q_r = S_8*C_r, P_r = C_8*C_r  (r=1..8)
    using sin((8+r)x) = 2*Q_r - sin((8-r)x), cos((8+r)x) = 2*P_r - cos((8-r)x);
    the linear reconstruction is absorbed into the weights on the host.
  - g=8 runs first so products interleave with seeds: per step the PE eats
    4 feature tiles (S_r, C_r, Q_r, P_r) while ACT makes 2 and DVE makes 2,
    keeping every engine below the PE rate.
  - y.T = W'.T @ F via PE (bf16, K=4096 accumulated in PSUM over 32 k-tiles).
  - PSUM drained on ACT (Identity + per-partition bias) to bf16.
  - Host: transpose/shard x, build W' from fouriercoeffs (fp64), assemble y.
"""
import math
import numpy as np
from contextlib import ExitStack

import concourse.bass as bass
import concourse.mybir as mybir
import concourse.tile as tile
from concourse import bacc
from concourse.bass_utils import run_bass_kernel_spmd

import ml_dtypes

N_CORES = 8
N_TOTAL = 32768
N_SHARD = N_TOTAL // N_CORES        # 4096 rows per core
INDIM = 128
OUTDIM = 256
GRID = 16
NFEAT = 32                          # 2*GRID features per i
SP = 4                              # n-superpasses per core
S = N_SHARD // SP                   # 2048 columns per superpass
CH = 512                            # matmul moving chunk (PSUM bank)
TWO_PI = 2.0 * math.pi

FP32 = mybir.dt.float32
BF16 = mybir.dt.bfloat16
I32 = mybir.dt.int32

# feature k-tile order (products interleaved with seeds):
#   kt0 = S_8, kt1 = C_8
#   for r=1..7: kt(4r-2)=S_r, kt(4r-1)=C_r, kt(4r)=Q_r, kt(4r+1)=P_r
#   kt30 = Q_8, kt31 = P_8
KT_S = {8: 0, **{r: 4 * r - 2 for r in range(1, 8)}}
KT_C = {8: 1, **{r: 4 * r - 1 for r in range(1, 8)}}
KT_Q = {**{r: 4 * r for r in range(1, 8)}, 8: 30}
KT_P = {**{r: 4 * r + 1 for r in range(1, 8)}, 8: 31}

_CACHED = {}


def _build():
    if "nc" in _CACHED:
        return _CACHED["nc"]
    nc = bacc.Bacc("TRN2", target_bir_lowering=False, debug=False,
                   num_devices=N_CORES)
    xt_d = nc.dram_tensor("xt", [INDIM, N_SHARD], FP32, kind="ExternalInput").ap()
    w_d = nc.dram_tensor("w", [INDIM, NFEAT * OUTDIM], BF16,
                         kind="ExternalInput").ap()
    bt_d = nc.dram_tensor("bt", [INDIM, 2], FP32, kind="ExternalInput").ap()
    bias_d = nc.dram_tensor("bias", [INDIM, 2], FP32, kind="ExternalInput").ap()
    yt_d = nc.dram_tensor("yt", [OUTDIM, N_SHARD], BF16, kind="ExternalOutput").ap()

    with tile.TileContext(nc) as tc, ExitStack() as ctx:
        cpool = ctx.enter_context(tc.tile_pool(name="const", bufs=1))
        vpool = ctx.enter_context(tc.tile_pool(name="v", bufs=1))
        rpool = ctx.enter_context(tc.tile_pool(name="r", bufs=2))
        base8 = ctx.enter_context(tc.tile_pool(name="base8", bufs=2))
        seedpool = ctx.enter_context(tc.tile_pool(name="seed", bufs=2))
        prodpool = ctx.enter_context(tc.tile_pool(name="prod", bufs=3))
        ypool = ctx.enter_context(tc.tile_pool(name="y", bufs=1))
        ppool = ctx.enter_context(tc.tile_pool(name="psum", bufs=1, space="PSUM"))

        # first x slice leads (longest dependency chain); small consts are
        # quick DIRECT2D transfers right behind it
        xt = cpool.tile([INDIM, N_SHARD], FP32)
        wt = cpool.tile([INDIM, NFEAT * OUTDIM], BF16)
        nc.sync.dma_start(xt[:, 0:512], xt_d[:, 0:512])
        bt = cpool.tile([INDIM, 2], FP32)
        nc.sync.dma_start(bt[:], bt_d[:])
        bias = cpool.tile([INDIM, 2], FP32)
        nc.sync.dma_start(bias[:], bias_d[:])
        nc.sync.dma_start(wt[:, 0:512], w_d[:, 0:512])          # kt 0-1
        nc.sync.dma_start(xt[:, 512:2048], xt_d[:, 512:2048])
        nc.sync.dma_start(wt[:, 512:2048], w_d[:, 512:2048])    # kt 2-7
        nc.sync.dma_start(xt[:, 2048:N_SHARD], xt_d[:, 2048:N_SHARD])
        nc.sync.dma_start(wt[:, 2048:NFEAT * OUTDIM],
                          w_d[:, 2048:NFEAT * OUTDIM])

        def mm_feature(kt, f, psums):
            """Accumulate feature tile f (k-tile kt) into both output halves."""
            for oh in range(2):
                lhsT = wt[:, kt * OUTDIM + oh * 128:kt * OUTDIM + oh * 128 + 128]
                for chi in range(S // CH):
                    nc.tensor.matmul(
                        psums[oh][:, chi * CH:(chi + 1) * CH],
                        lhsT, f[:, chi * CH:(chi + 1) * CH],
                        start=(kt == 0), stop=(kt == NFEAT - 1),
                    )

        for sp in range(SP):
            xs = xt[:, sp * S:(sp + 1) * S]
            psum0 = ppool.tile([128, S], FP32, tag=f"p0{sp % 2}")
            psum1 = ppool.tile([128, S], FP32, tag=f"p1{sp % 2}")
            psums = [psum0, psum1]

            def make_seed(g, sg, cg, nsec=1):
                a = np.float32(g / TWO_PI)
                p = np.float32(TWO_PI / g)
                sec = S // nsec
                for si in range(nsec):
                    sl = slice(si * sec, (si + 1) * sec)
                    v = vpool.tile([INDIM, sec], I32, tag="v")
                    nc.vector.tensor_scalar(v[:], xs[:, sl], float(a), 0.125,
                                            mybir.AluOpType.mult,
                                            mybir.AluOpType.add)
                    r = rpool.tile([INDIM, sec], FP32, tag="r")
                    nc.vector.scalar_tensor_tensor(r[:], v[:], float(-p),
                                                   xs[:, sl],
                                                   mybir.AluOpType.mult,
                                                   mybir.AluOpType.add)
                    nc.scalar.activation(sg[:, sl], r[:],
                                         mybir.ActivationFunctionType.Sin,
                                         bias=bt[:, 0:1], scale=float(g))
                    nc.scalar.activation(cg[:, sl], r[:],
                                         mybir.ActivationFunctionType.Sin,
                                         bias=bt[:, 1:2], scale=float(g))

            def drain(oh, nchunks=1):
                dc = S // nchunks
                for ci in range(nchunks):
                    y = ypool.tile([128, dc], BF16, tag=f"y{oh}{ci}{nchunks}")
                    nc.scalar.activation(y[:], psums[oh][:, ci * dc:(ci + 1) * dc],
                                         mybir.ActivationFunctionType.Identity,
                                         bias=bias[:, oh:oh + 1], scale=1.0)
                    nc.sync.dma_start(
                        yt_d[oh * 128:(oh + 1) * 128,
                             sp * S + ci * dc:sp * S + (ci + 1) * dc], y[:])

            s8 = base8.tile([INDIM, S], BF16, tag="S8")
            c8 = base8.tile([INDIM, S], BF16, tag="C8")
            make_seed(8, s8, c8, nsec=2 if sp == 0 else 1)
            mm_feature(KT_S[8], s8, psums)
            mm_feature(KT_C[8], c8, psums)

            for r_ in range(1, 8):
                sg = seedpool.tile([INDIM, S], BF16, tag="Sr")
                cg = seedpool.tile([INDIM, S], BF16, tag="Cr")
                make_seed(r_, sg, cg)
                mm_feature(KT_S[r_], sg, psums)
                mm_feature(KT_C[r_], cg, psums)
                q = prodpool.tile([INDIM, S], BF16, tag="q")
                nc.vector.tensor_tensor(q[:], s8[:], cg[:],
                                        mybir.AluOpType.mult)
                mm_feature(KT_Q[r_], q, psums)
                pr = prodpool.tile([INDIM, S], BF16, tag="p")
                nc.vector.tensor_tensor(pr[:], c8[:], cg[:],
                                        mybir.AluOpType.mult)
                mm_feature(KT_P[r_], pr, psums)

            q = prodpool.tile([INDIM, S], BF16, tag="q")
            nc.vector.tensor_tensor(q[:], s8[:], c8[:], mybir.AluOpType.mult)
            mm_feature(KT_Q[8], q, psums)
            pr = prodpool.tile([INDIM, S], BF16, tag="p")
            nc.vector.tensor_tensor(pr[:], c8[:], c8[:], mybir.AluOpType.mult)
            # last k-tile: finish oh0, drain it while oh1 finishes
            kt = KT_P[8]
            for oh in range(2):
                lhsT = wt[:, kt * OUTDIM + oh * 128:kt * OUTDIM + oh * 128 + 128]
                for chi in range(S // CH):
                    nc.tensor.matmul(
                        psums[oh][:, chi * CH:(chi + 1) * CH],
                        lhsT, pr[:, chi * CH:(chi + 1) * CH],
                        start=False, stop=True,
                    )
                drain(oh, nchunks=1)

    nc.compile()
    _CACHED["nc"] = nc
    return nc


def _prep_inputs(x: np.ndarray, fouriercoeffs: np.ndarray, bias: np.ndarray):
    xt = np.ascontiguousarray(x.astype(np.float32, copy=False).T)  # (128, 32768)

    fc = fouriercoeffs.astype(np.float64, copy=False)
    c_cos = fc[0]                     # (256 o, 128 i, 16 g): cos coeffs
    c_sin = fc[1]                     # sin coeffs

    # sin((8+r)x) = 2 Q_r - sin((8-r)x)   [sin(0x) = 0]
    # cos((8+r)x) = 2 P_r - cos((8-r)x)   [cos(0x) = 1 -> bias]
    wf = np.zeros((NFEAT, OUTDIM, INDIM), np.float64)
    for g in range(1, 9):
        wf[KT_S[g]] += c_sin[:, :, g - 1]
        wf[KT_C[g]] += c_cos[:, :, g - 1]
    for r in range(1, 9):
        gs = 8 + r
        wf[KT_Q[r]] += 2.0 * c_sin[:, :, gs - 1]
        wf[KT_P[r]] += 2.0 * c_cos[:, :, gs - 1]
        if r < 8:
            wf[KT_S[8 - r]] -= c_sin[:, :, gs - 1]
            wf[KT_C[8 - r]] -= c_cos[:, :, gs - 1]
    bias_eff = bias.astype(np.float64).reshape(OUTDIM).copy()
    bias_eff -= c_cos[:, :, 15].sum(axis=1)                   # cos(0x)*W_c16

    # SBUF weight tile: [i, kt*256 + o] bf16
    w_sb = np.ascontiguousarray(
        wf.transpose(2, 0, 1).reshape(INDIM, NFEAT * OUTDIM)
    ).astype(ml_dtypes.bfloat16)

    bt = np.tile(np.array([[0.0, math.pi / 2.0]], dtype=np.float32), (INDIM, 1))
    bias_sb = np.ascontiguousarray(
        bias_eff.reshape(2, 128).T.astype(np.float32))        # (128, 2)
    return xt, w_sb, bt, bias_sb


def kernel(x: np.ndarray, fouriercoeffs: np.ndarray, bias: np.ndarray,
           _trace: bool = False):
    x = np.asarray(x)
    fouriercoeffs = np.asarray(fouriercoeffs)
    bias = np.asarray(bias)
    orig_shape = x.shape
    x2 = x.reshape(-1, INDIM)
    assert x2.shape == (N_TOTAL, INDIM), x2.shape

    nc = _build()
    xt, w_sb, bt, bias_sb = _prep_inputs(x2, fouriercoeffs, bias)
    in_maps = []
    for c in range(N_CORES):
        in_maps.append({
            "xt": np.ascontiguousarray(xt[:, c * N_SHARD:(c + 1) * N_SHARD]),
            "w": w_sb,
            "bt": bt,
            "bias": bias_sb,
        })
    res = run_bass_kernel_spmd(nc, in_maps, list(range(N_CORES)),
                               trace=_trace)
    yt = np.concatenate([res.results[c]["yt"].astype(np.float32)
                         for c in range(N_CORES)], axis=1)
    y = np.ascontiguousarray(yt.T)
    if _trace:
        kernel._last_result = res
    return y.reshape(*orig_shape[:-1], OUTDIM)


# revision 29
# speedup vs baseline: 1.1840x; 1.0050x over previous
"""KAN Fourier-linear kernel for 8 Trainium2 NeuronCores.

y[n,o] = sum_{i,g} C0[o,i,g]*cos(g*x[n,i]) + C1[o,i,g]*sin(g*x[n,i]) + bias[o]

Strategy (data-parallel over n, 4096 rows/core, 2 superpasses of 2048):
  - Seeds g=1..8 computed exactly:
      v   = rne(x*(g/2pi) + 1/8)          (DVE tensor_scalar fp32->int32)
      r   = x - v*(2pi/g)                 (DVE scalar_tensor_tensor fp32)
      S_g = Sin(scale=g, bias=0)(r)       = sin(g*x)   (ACT, arg in [-3.93, 2.36])
      C_g = Sin(scale=g, bias=pi/2)(r)    = cos(g*x)   (ACT, arg in [-2.36, 3.93])
  - g=9..16 as product features on DVE (TT bf16, 2x mode):
      Q_r = S_8*C_r, P_r = C_8*C_r  (r=1..8)
    using sin((8+r)x) = 2*Q_r - sin((8-r)x), cos((8+r)x) = 2*P_r - cos((8-r)x);
    the linear reconstruction is absorbed into the weights on the host.
  - g=8 runs first so products interleave with seeds: per step the PE eats
    4 feature tiles (S_r, C_r, Q_r, P_r) while ACT makes 2 and DVE makes 2,
    keeping every engine below the PE rate.
  - y.T = W'.T @ F via PE (bf16, K=4096 accumulated in PSUM over 32 k-tiles).
  - PSUM drained on ACT (Identity + per-partition bias) to bf16.
  - Host: transpose/shard x, build W' from fouriercoeffs (fp64), assemble y.
"""
import math
import numpy as np
from contextlib import ExitStack

import concourse.bass as bass
import concourse.mybir as mybir
import concourse.tile as tile
from concourse import bacc
from concourse.bass_utils import run_bass_kernel_spmd

import ml_dtypes

N_CORES = 8
N_TOTAL = 32768
N_SHARD = N_TOTAL // N_CORES        # 4096 rows per core
INDIM = 128
OUTDIM = 256
GRID = 16
NFEAT = 32                          # 2*GRID features per i
SP = 4                              # n-superpasses per core
S = N_SHARD // SP                   # 2048 columns per superpass
CH = 512                            # matmul moving chunk (PSUM bank)
TWO_PI = 2.0 * math.pi

FP32 = mybir.dt.float32
BF16 = mybir.dt.bfloat16
I32 = mybir.dt.int32

# feature k-tile order (products interleaved with seeds):
#   kt0 = S_8, kt1 = C_8
#   for r=1..7: kt(4r-2)=S_r, kt(4r-1)=C_r, kt(4r)=Q_r, kt(4r+1)=P_r
#   kt30 = Q_8, kt31 = P_8
KT_S = {8: 0, **{r: 4 * r - 2 for r in range(1, 8)}}
KT_C = {8: 1, **{r: 4 * r - 1 for r in range(1, 8)}}
KT_Q = {**{r: 4 * r for r in range(1, 8)}, 8: 30}
KT_P = {**{r: 4 * r + 1 for r in range(1, 8)}, 8: 31}

_CACHED = {}


def _build():
    if "nc" in _CACHED:
        return _CACHED["nc"]
    nc = bacc.Bacc("TRN2", target_bir_lowering=False, debug=False,
                   num_devices=N_CORES)
    xt_d = nc.dram_tensor("xt", [INDIM, N_SHARD], FP32, kind="ExternalInput").ap()
    w_d = nc.dram_tensor("w", [INDIM, NFEAT * OUTDIM], BF16,
                         kind="ExternalInput").ap()
    bt_d = nc.dram_tensor("bt", [INDIM, 2], FP32, kind="ExternalInput").ap()
    bias_d = nc.dram_tensor("bias", [INDIM, 2], FP32, kind="ExternalInput").ap()
    yt_d = nc.dram_tensor("yt", [OUTDIM, N_SHARD], BF16, kind="ExternalOutput").ap()

    with tile.TileContext(nc) as tc, ExitStack() as ctx:
        cpool = ctx.enter_context(tc.tile_pool(name="const", bufs=1))
        vpool = ctx.enter_context(tc.tile_pool(name="v", bufs=1))
        rpool = ctx.enter_context(tc.tile_pool(name="r", bufs=2))
        base8 = ctx.enter_context(tc.tile_pool(name="base8", bufs=2))
        seedpool = ctx.enter_context(tc.tile_pool(name="seed", bufs=2))
        prodpool = ctx.enter_context(tc.tile_pool(name="prod", bufs=3))
        ypool = ctx.enter_context(tc.tile_pool(name="y", bufs=1))
        ppool = ctx.enter_context(tc.tile_pool(name="psum", bufs=1, space="PSUM"))

        # first x slice leads (longest dependency chain); small consts are
        # quick DIRECT2D transfers right behind it
        xt = cpool.tile([INDIM, N_SHARD], FP32)
        wt = cpool.tile([INDIM, NFEAT * OUTDIM], BF16)
        nc.sync.dma_start(xt[:, 0:512], xt_d[:, 0:512])
        nc.sync.dma_start(wt[:, 0:512], w_d[:, 0:512])          # kt 0-1
        bt = cpool.tile([INDIM, 2], FP32)
        nc.sync.dma_start(bt[:], bt_d[:])
        bias = cpool.tile([INDIM, 2], FP32)
        nc.sync.dma_start(bias[:], bias_d[:])
        nc.sync.dma_start(xt[:, 512:2048], xt_d[:, 512:2048])
        nc.sync.dma_start(wt[:, 512:2048], w_d[:, 512:2048])    # kt 2-7
        nc.sync.dma_start(xt[:, 2048:N_SHARD], xt_d[:, 2048:N_SHARD])
        nc.sync.dma_start(wt[:, 2048:NFEAT * OUTDIM],
                          w_d[:, 2048:NFEAT * OUTDIM])

        def mm_feature(kt, f, psums):
            """Accumulate feature tile f (k-tile kt) into both output halves."""
            for oh in range(2):
                lhsT = wt[:, kt * OUTDIM + oh * 128:kt * OUTDIM + oh * 128 + 128]
                for chi in range(S // CH):
                    nc.tensor.matmul(
                        psums[oh][:, chi * CH:(chi + 1) * CH],
                        lhsT, f[:, chi * CH:(chi + 1) * CH],
                        start=(kt == 0), stop=(kt == NFEAT - 1),
                    )

        for sp in range(SP):
            xs = xt[:, sp * S:(sp + 1) * S]
            psum0 = ppool.tile([128, S], FP32, tag=f"p0{sp % 2}")
            psum1 = ppool.tile([128, S], FP32, tag=f"p1{sp % 2}")
            psums = [psum0, psum1]

            def make_seed(g, sg, cg, nsec=1):
                a = np.float32(g / TWO_PI)
                p = np.float32(TWO_PI / g)
                sec = S // nsec
                for si in range(nsec):
                    sl = slice(si * sec, (si + 1) * sec)
                    v = vpool.tile([INDIM, sec], I32, tag="v")
                    nc.vector.tensor_scalar(v[:], xs[:, sl], float(a), 0.125,
                                            mybir.AluOpType.mult,
                                            mybir.AluOpType.add)
                    r = rpool.tile([INDIM, sec], FP32, tag="r")
                    nc.vector.scalar_tensor_tensor(r[:], v[:], float(-p),
                                                   xs[:, sl],
                                                   mybir.AluOpType.mult,
                                                   mybir.AluOpType.add)
                    nc.scalar.activation(sg[:, sl], r[:],
                                         mybir.ActivationFunctionType.Sin,
                                         bias=bt[:, 0:1], scale=float(g))
                    nc.scalar.activation(cg[:, sl], r[:],
                                         mybir.ActivationFunctionType.Sin,
                                         bias=bt[:, 1:2], scale=float(g))

            def drain(oh, nchunks=1):
                dc = S // nchunks
                for ci in range(nchunks):
                    y = ypool.tile([128, dc], BF16, tag=f"y{oh}{ci}{nchunks}")
                    nc.scalar.activation(y[:], psums[oh][:, ci * dc:(ci + 1) * dc],
                                         mybir.ActivationFunctionType.Identity,
                                         bias=bias[:, oh:oh + 1], scale=1.0)
                    nc.sync.dma_start(
                        yt_d[oh * 128:(oh + 1) * 128,
                             sp * S + ci * dc:sp * S + (ci + 1) * dc], y[:])

            s8 = base8.tile([INDIM, S], BF16, tag="S8")
            c8 = base8.tile([INDIM, S], BF16, tag="C8")
            make_seed(8, s8, c8, nsec=2 if sp == 0 else 1)
            mm_feature(KT_S[8], s8, psums)
            mm_feature(KT_C[8], c8, psums)

            for r_ in range(1, 8):
                sg = seedpool.tile([INDIM, S], BF16, tag="Sr")
                cg = seedpool.tile([INDIM, S], BF16, tag="Cr")
                make_seed(r_, sg, cg)
                mm_feature(KT_S[r_], sg, psums)
                mm_feature(KT_C[r_], cg, psums)
                q = prodpool.tile([INDIM, S], BF16, tag="q")
                nc.vector.tensor_tensor(q[:], s8[:], cg[:],
                                        mybir.AluOpType.mult)
                mm_feature(KT_Q[r_], q, psums)
                pr = prodpool.tile([INDIM, S], BF16, tag="p")
                nc.vector.tensor_tensor(pr[:], c8[:], cg[:],
                                        mybir.AluOpType.mult)
                mm_feature(KT_P[r_], pr, psums)

            q = prodpool.tile([INDIM, S], BF16, tag="q")
            nc.vector.tensor_tensor(q[:], s8[:], c8[:], mybir.AluOpType.mult)
            mm_feature(KT_Q[8], q, psums)
            pr = prodpool.tile([INDIM, S], BF16, tag="p")
            nc.vector.tensor_tensor(pr[:], c8[:], c8[:], mybir.AluOpType.mult)
            # last k-tile: finish oh0, drain it while oh1 finishes
            kt = KT_P[8]
            for oh in range(2):
                lhsT = wt[:, kt * OUTDIM + oh * 128:kt * OUTDIM + oh * 128 + 128]
                for chi in range(S // CH):
                    nc.tensor.matmul(
                        psums[oh][:, chi * CH:(chi + 1) * CH],
                        lhsT, pr[:, chi * CH:(chi + 1) * CH],
                        start=False, stop=True,
                    )
                drain(oh, nchunks=1)

    nc.compile()
    _CACHED["nc"] = nc
    return nc


def _prep_inputs(x: np.ndarray, fouriercoeffs: np.ndarray, bias: np.ndarray):
    xt = np.ascontiguousarray(x.astype(np.float32, copy=False).T)  # (128, 32768)

    fc = fouriercoeffs.astype(np.float64, copy=False)
    c_cos = fc[0]                     # (256 o, 128 i, 16 g): cos coeffs
    c_sin = fc[1]                     # sin coeffs

    # sin((8+r)x) = 2 Q_r - sin((8-r)x)   [sin(0x) = 0]
    # cos((8+r)x) = 2 P_r - cos((8-r)x)   [cos(0x) = 1 -> bias]
    wf = np.zeros((NFEAT, OUTDIM, INDIM), np.float64)
    for g in range(1, 9):
        wf[KT_S[g]] += c_sin[:, :, g - 1]
        wf[KT_C[g]] += c_cos[:, :, g - 1]
    for r in range(1, 9):
        gs = 8 + r
        wf[KT_Q[r]] += 2.0 * c_sin[:, :, gs - 1]
        wf[KT_P[r]] += 2.0 * c_cos[:, :, gs - 1]
        if r < 8:
            wf[KT_S[8 - r]] -= c_sin[:, :, gs - 1]
            wf[KT_C[8 - r]] -= c_cos[:, :, gs - 1]
    bias_eff = bias.astype(np.float64).reshape(OUTDIM).copy()
    bias_eff -= c_cos[:, :, 15].sum(axis=1)                   # cos(0x)*W_c16

    # SBUF weight tile: [i, kt*256 + o] bf16
    w_sb = np.ascontiguousarray(
        wf.transpose(2, 0, 1).reshape(INDIM, NFEAT * OUTDIM)
    ).astype(ml_dtypes.bfloat16)

    bt = np.tile(np.array([[0.0, math.pi / 2.0]], dtype=np.float32), (INDIM, 1))
    bias_sb = np.ascontiguousarray(
        bias_eff.reshape(2, 128).T.astype(np.float32))        # (128, 2)
    return xt, w_sb, bt, bias_sb


def kernel(x: np.ndarray, fouriercoeffs: np.ndarray, bias: np.ndarray,
           _trace: bool = False):
    x = np.asarray(x)
    fouriercoeffs = np.asarray(fouriercoeffs)
    bias = np.asarray(bias)
    orig_shape = x.shape
    x2 = x.reshape(-1, INDIM)
    assert x2.shape == (N_TOTAL, INDIM), x2.shape

    nc = _build()
    xt, w_sb, bt, bias_sb = _prep_inputs(x2, fouriercoeffs, bias)
    in_maps = []
    for c in range(N_CORES):
        in_maps.append({
            "xt": np.ascontiguousarray(xt[:, c * N_SHARD:(c + 1) * N_SHARD]),
            "w": w_sb,
            "bt": bt,
            "bias": bias_sb,
        })
    res = run_bass_kernel_spmd(nc, in_maps, list(range(N_CORES)),
                               trace=_trace)
    yt = np.concatenate([res.results[c]["yt"].astype(np.float32)
                         for c in range(N_CORES)], axis=1)
    y = np.ascontiguousarray(yt.T)
    if _trace:
        kernel._last_result = res
    return y.reshape(*orig_shape[:-1], OUTDIM)
